# revision 5
# baseline (speedup 1.0000x reference)
"""ConsistencyLoss Trainium2 kernel (single device phase).

Problem: B=16 depth frames, 15 consecutive pairs. Per pair: unproject
depth A, rigid-transform into frame B, project+round, z-buffer
scatter-min into B's grid, compare with depth B -> scalar loss; summed.

Sharding: data-parallel over pairs, 2 pairs per core across 8 cores
(core 7 duplicates pair 13 in slot 0; host ignores it).

Device (one launch, per core, 12 row-chunks of 128x1024):
with r = 1/d, the projection is u2 = Nx/Nz, v2 = Ny/Nz, z = d*Nz where
  N_* = A_*.u + c_*(v) + T_*.r      (affine field + reciprocal term)
Holes (d=0) self-eliminate: r=inf -> Nz=+-inf -> u2=NaN/huge -> the
rounded u16 coord falls outside [1,1024]. z<=0 is killed by relu(Nz)
before the 1/Nz reciprocal (1/0=inf -> coords saturate out of range).
Rounding uses the +-2^23 RNE trick (matches jnp.round); coords are
emitted +1-shifted as fp16 (exact for integers <=2048), z as fp16.
Work is split DVE / Activation / GPSIMD roughly 7/5/6 us per chunk.
Per-frame nonzero counts (cnt denominators) come from an
is_finite(1/d) accumulation on the Activation engine.

Host: the per-pair scatter-min combine - u64 key sort ((idx<<16)|zbits,
fp16 bit order = value order for z>0) - plus the O(hits) loss assembly
S = sum(zmin) - sum(dB[hit]), cnt = nb(dB) + hits - nb_hit. This stays
on the host because TRN2 has no per-element scatter primitive
(indirect DMA RMW races lose duplicate updates; gpsimd scatter ops
share indices across partitions).
"""
import os
import sys

try:
    import concourse.bass as bass
except ImportError:
    sys.path.insert(0, "/opt/trn_rl_repo")
    import concourse.bass as bass

import numpy as np
import concourse.mybir as mybir
from concourse.bass_utils import run_bass_kernel_spmd

f32 = mybir.dt.float32
f16 = mybir.dt.float16
Alu = mybir.AluOpType
Act = mybir.ActivationFunctionType

B, H, W = 16, 768, 1024
NPAIR = B - 1          # 15
NCORE = 8
CHUNKS = H // 128      # 6
NCH = 2 * CHUNKS       # 12
M23 = float(1.5 * 2.0 ** 23)     # RNE rounding magic
BIAS1 = M23 + 1.0                # round + 1-shift in one add

LAST_PROFILE = {}


def _trace_enabled():
    return os.environ.get("CONSISTENCY_TRACE", "0") == "1"


def _quat_to_rot(q):
    q = q / np.linalg.norm(q)
    x, y, z, w = q
    return np.array([
        [1 - 2 * (y * y + z * z), 2 * (x * y - z * w), 2 * (x * z + y * w)],
        [2 * (x * y + z * w), 1 - 2 * (x * x + z * z), 2 * (y * z - x * w)],
        [2 * (x * z - y * w), 2 * (y * z + x * w), 1 - 2 * (x * x + y * y)],
    ])


def build_kernel():
    nc = bass.Bass()
    frames = nc.declare_dram_parameter("frames", [2, H, W], f32, isOutput=False)
    coefs = nc.declare_dram_parameter("coefs", [128, 48], f32, isOutput=False)
    uramp = nc.declare_dram_parameter("uramp", [128, W], f32, isOutput=False)
    out = nc.declare_dram_parameter("out", [2, H, 3 * W], f16, isOutput=True)
    nbacc = nc.declare_dram_parameter("nbacc", [128, NCH], f32, isOutput=True)

    import contextlib
    with contextlib.ExitStack() as stack:
        en = stack.enter_context
        d = en(nc.sbuf_tensor([128, 4 * W], f32))      # input depth, 4 slots
        rb = en(nc.sbuf_tensor([128, 2 * W], f32))     # 1/d, 2 slots
        nzb = en(nc.sbuf_tensor([128, 2 * W], f32))    # Nz
        nzpb = en(nc.sbuf_tensor([128, 2 * W], f32))   # relu(Nz)
        u2b = en(nc.sbuf_tensor([128, 2 * W], f32))    # Nx/Nz
        nyb = en(nc.sbuf_tensor([128, 2 * W], f32))    # Ny
        rzb = en(nc.sbuf_tensor([128, 2 * W], f32))    # 1/relu(Nz)
        fzb = en(nc.sbuf_tensor([128, 2 * W], f32))    # field z
        fxb = en(nc.sbuf_tensor([128, 2 * W], f32))    # field x
        fyb = en(nc.sbuf_tensor([128, 2 * W], f32))    # field y
        nxs = en(nc.sbuf_tensor([128, W], f32))        # Nx (DVE only)
        v2s = en(nc.sbuf_tensor([128, W], f32))        # v2 (Pool only)
        scr = en(nc.sbuf_tensor([128, W], f32))        # ACT dump
        ob = en(nc.sbuf_tensor([128, 2 * 3 * W], f16))  # out tile, 2 slots
        ur = en(nc.sbuf_tensor([128, W], f32))         # u ramp
        co = en(nc.sbuf_tensor([128, 48], f32))        # coefficients
        nb = en(nc.sbuf_tensor([128, NCH], f32))       # nonzero count accum
        dsem = en(nc.semaphore())
        osem = en(nc.semaphore())
        vsem = en(nc.semaphore())
        asem = en(nc.semaphore())
        psem = en(nc.semaphore())
        block = en(nc.Block())
        def dsl(k):
            b = (k % 4) * W
            return d[:, b:b + W]

        def sl(t, k, w=W):
            b = (k % 2) * w
            return t[:, b:b + w]

        def orow(k, i):
            b = (k % 2) * 3 * W + i * W
            return ob[:, b:b + W]

        def ccol(t, s, i):
            c = 24 * s + i
            return t[:, c:c + 1]

        # coefficient columns per pair s: 0-5 czv, 6-11 cxv, 12-17 cyv,
        # 18 Az, 19 Ax, 20 Ay, 21 tz, 22 TX, 23 TY
        # sem counts: vsem (DVE): r@5k+1, Nz@5k+2, Ny@5k+3, rz@5k+4, u2@5k+5
        #             asem (ACT): pre 6; relu@5k+7, nb@5k+8, f*@5k+9..11
        #             psem (Pool): v2@3k+1, zt@3k+2, uc@3k+3; dma: dsem/osem
        @block.gpsimd
        def _(g):
            g.dma_start(ur[:], uramp[:]).then_inc(dsem, 16)
            g.dma_start(co[:], coefs[:]).then_inc(dsem, 16)
            for k in range(2):
                s, j = divmod(k, CHUNKS)
                g.dma_start(dsl(k), frames[s, 128 * j:128 * j + 128]
                            ).then_inc(dsem, 16)
            for k in range(NCH):
                s, j = divmod(k, CHUNKS)
                if k + 2 < NCH:
                    s2, j2 = divmod(k + 2, CHUNKS)
                    g.dma_start(dsl(k + 2), frames[s2, 128 * j2:128 * j2 + 128]
                                ).then_inc(dsem, 16)
                # v2 = Ny * (1/relu(Nz))
                g.wait_ge(vsem, 5 * k + 4)
                nc.gpsimd.tensor_tensor(
                    v2s[:], sl(nyb, k), sl(rzb, k), Alu.mult).then_inc(psem, 1)
                # z = d * Nz -> fp16
                if k >= 2:
                    g.wait_ge(osem, 16 * (k - 1))
                nc.gpsimd.tensor_tensor(
                    orow(k, 2), dsl(k), sl(nzb, k), Alu.mult).then_inc(psem, 1)
                # vc/uc = round(x)+1 as fp16 (RNE via +-2^23)
                nc.gpsimd.tensor_scalar(
                    orow(k, 1), v2s[:], BIAS1, M23, Alu.add, Alu.subtract)
                g.wait_ge(vsem, 5 * k + 5)
                nc.gpsimd.tensor_scalar(
                    orow(k, 0), sl(u2b, k), BIAS1, M23, Alu.add, Alu.subtract
                ).then_inc(psem, 1)
                g.dma_start(out[s, 128 * j:128 * j + 128], sl(ob, k, 3 * W)
                            ).then_inc(osem, 16)
            g.wait_ge(asem, 6 + 5 * NCH)
            g.dma_start(nbacc[:], nb[:]).then_inc(osem, 16)

        @block.vector
        def _(v):
            for k in range(NCH):
                s, j = divmod(k, CHUNKS)
                v.wait_ge(dsem, 16 * (k + 3))
                nc.vector.reciprocal(sl(rb, k), dsl(k)).then_inc(vsem, 1)
                v.wait_ge(asem, 5 * k - 1 if k >= 2 else 3 * k + 1)
                if k >= 2:
                    v.wait_ge(psem, 3 * (k - 2) + 2)
                nc.vector.scalar_tensor_tensor(
                    sl(nzb, k), sl(rb, k), ccol(co, s, 21), sl(fzb, k),
                    Alu.mult, Alu.add).then_inc(vsem, 1)
                v.wait_ge(asem, 5 * k if k >= 2 else 3 * k + 2)
                nc.vector.scalar_tensor_tensor(
                    nxs[:], sl(rb, k), ccol(co, s, 22), sl(fxb, k),
                    Alu.mult, Alu.add)
                v.wait_ge(asem, 5 * k + 1 if k >= 2 else 3 * k + 3)
                nc.vector.scalar_tensor_tensor(
                    sl(nyb, k), sl(rb, k), ccol(co, s, 23), sl(fyb, k),
                    Alu.mult, Alu.add).then_inc(vsem, 1)
                v.wait_ge(asem, 5 * k + 7)
                nc.vector.reciprocal(sl(rzb, k), sl(nzpb, k)).then_inc(vsem, 1)
                if k >= 2:
                    v.wait_ge(psem, 3 * (k - 2) + 3)
                nc.vector.tensor_tensor(
                    sl(u2b, k), nxs[:], sl(rzb, k), Alu.mult).then_inc(vsem, 1)

        @block.scalar
        def _(a):
            a.wait_ge(dsem, 32)
            for k in range(2):
                s, j = divmod(k, CHUNKS)
                nc.scalar.activation(sl(fzb, k), ur[:], Act.Identity,
                                     bias=ccol(co, s, j), scale=ccol(co, s, 18)
                                     ).then_inc(asem, 1)
                nc.scalar.activation(sl(fxb, k), ur[:], Act.Identity,
                                     bias=ccol(co, s, 6 + j), scale=ccol(co, s, 19)
                                     ).then_inc(asem, 1)
                nc.scalar.activation(sl(fyb, k), ur[:], Act.Identity,
                                     bias=ccol(co, s, 12 + j), scale=ccol(co, s, 20)
                                     ).then_inc(asem, 1)
            for k in range(NCH):
                a.wait_ge(vsem, 5 * k + 2)
                nc.scalar.activation(sl(nzpb, k), sl(nzb, k), Act.Relu
                                     ).then_inc(asem, 1)
                # per-frame nonzero count: is_finite(1/d), accumulated
                nc.scalar.activation(scr[:], sl(rb, k), Act.Is_finite,
                                     accum_out=nb[:, k:k + 1]).then_inc(asem, 1)
                kk = (k + 2) % NCH
                s3, j3 = divmod(kk, CHUNKS)
                a.wait_ge(vsem, 5 * k + 4)
                nc.scalar.activation(sl(fzb, kk), ur[:], Act.Identity,
                                     bias=ccol(co, s3, j3), scale=ccol(co, s3, 18)
                                     ).then_inc(asem, 1)
                nc.scalar.activation(sl(fxb, kk), ur[:], Act.Identity,
                                     bias=ccol(co, s3, 6 + j3), scale=ccol(co, s3, 19)
                                     ).then_inc(asem, 1)
                nc.scalar.activation(sl(fyb, kk), ur[:], Act.Identity,
                                     bias=ccol(co, s3, 12 + j3), scale=ccol(co, s3, 20)
                                     ).then_inc(asem, 1)
    return nc


_NC = None


def _get_module():
    global _NC
    if _NC is None:
        _NC = build_kernel()
    return _NC


def _maybe_enable_hook():
    """Register the axon NTFF profile hook if the image lacks antenv."""
    if not _trace_enabled():
        return
    try:
        import types
        import antenv.axon_hooks  # noqa: F401
    except ImportError:
        try:
            import trn_agent_boot.trn_boot as tb
            hook = tb._ntff_profile_via_ctypes("/opt/axon/libaxon_pjrt.so")
            m = types.ModuleType("antenv.axon_hooks")
            m.get_axon_ntff_profile_hook = lambda: hook
            m.set_axon_ntff_profile_hook = lambda h: None
            pkg = sys.modules.get("antenv") or types.ModuleType("antenv")
            pkg.axon_hooks = m
            sys.modules.setdefault("antenv", pkg)
            sys.modules["antenv.axon_hooks"] = m
            import concourse.bass_utils as bu
            bu.upload_artifacts = lambda d: "local://" + str(d)
        except Exception:
            pass


STARTS = [0, 2, 4, 6, 8, 10, 12, 13]


def _make_coefs(pose, K):
    fx, fy, cx, cy = (float(K[0, 0]), float(K[1, 1]),
                      float(K[0, 2]), float(K[1, 2]))
    v = np.arange(H, dtype=np.float64)
    b_v = (v - cy) / fy
    all_coefs = []
    for c in range(NCORE):
        st = STARTS[c]
        co = np.zeros((128, 48), np.float32)
        for s in range(2):
            i = st + s
            RA = _quat_to_rot(pose[i, 3:].astype(np.float64))
            tA = pose[i, :3].astype(np.float64)
            RB = _quat_to_rot(pose[i + 1, 3:].astype(np.float64))
            tB = pose[i + 1, :3].astype(np.float64)
            M = RB.T @ RA
            tp = RB.T @ (tA - tB)
            rows = {
                'z': (M[2, 0], M[2, 1], M[2, 2], tp[2]),
                'x': (fx * M[0, 0] + cx * M[2, 0], fx * M[0, 1] + cx * M[2, 1],
                      fx * M[0, 2] + cx * M[2, 2], fx * tp[0] + cx * tp[2]),
                'y': (fy * M[1, 0] + cy * M[2, 0], fy * M[1, 1] + cy * M[2, 1],
                      fy * M[1, 2] + cy * M[2, 2], fy * tp[1] + cy * tp[2]),
            }
            for gi, key in enumerate(('z', 'x', 'y')):
                C0, C1, C2, C3 = rows[key]
                colv = (-C0 * cx / fx + C1 * b_v + C2).astype(np.float32)
                for j in range(CHUNKS):
                    co[:, 24 * s + 6 * gi + j] = colv[128 * j:128 * (j + 1)]
                co[:, 24 * s + 18 + gi] = np.float32(C0 / fx)
                co[:, 24 * s + 21 + gi] = np.float32(C3)
        all_coefs.append(co)
    return all_coefs


def kernel(pred, pose, K):
    pred = np.asarray(pred, dtype=np.float32)
    pose = np.asarray(pose, dtype=np.float32)
    K = np.asarray(K, dtype=np.float32)

    _maybe_enable_hook()
    nc = _get_module()

    all_coefs = _make_coefs(pose, K)
    urnp = np.broadcast_to(np.arange(W, dtype=np.float32), (128, W)).copy()
    in_maps = []
    for c in range(NCORE):
        st = STARTS[c]
        in_maps.append({
            "frames": np.ascontiguousarray(pred[st:st + 2, 0]),
            "coefs": all_coefs[c],
            "uramp": urnp,
        })

    res = run_bass_kernel_spmd(nc, in_maps, list(range(NCORE)),
                               trace=_trace_enabled())
    if res.exec_time_ns is not None:
        LAST_PROFILE["exec_ns"] = res.exec_time_ns

    # per-frame nonzero counts (frame f as some core's dA slot)
    nbcount = {}
    for c in range(NCORE):
        a = res.results[c]["nbacc"]
        for s in range(2):
            nbcount[STARTS[c] + s] = float(
                a[:, 6 * s:6 * (s + 1)].sum(dtype=np.float64))
    nbcount[B - 1] = float(np.count_nonzero(pred[B - 1, 0]))

    total = 0.0
    for p in range(NPAIR):
        if p == 14:
            c, s = 7, 1
        else:
            c, s = p // 2, p % 2
        o = res.results[c]["out"][s]          # [H, 3W] fp16
        uc = o[:, 0:W]
        vc = o[:, W:2 * W]
        zb = o[:, 2 * W:3 * W]
        with np.errstate(invalid='ignore'):
            ui = uc.astype(np.float64)
            vi = vc.astype(np.float64)
        zbits = zb.view(np.uint16).astype(np.int64)
        ok = ((ui >= 1) & (ui <= W) & (vi >= 1) & (vi <= H)
              & (zbits < 0x7C00) & (zbits > 0))
        idx = ((vi[ok] - 1).astype(np.int64) * W + (ui[ok] - 1).astype(np.int64))
        key = (idx << 16) | zbits[ok]
        key.sort()
        kidx = key >> 16
        first = np.ones(len(key), bool)
        first[1:] = kidx[1:] != kidx[:-1]
        widx = kidx[first]
        wz = ((key[first] & 0xFFFF).astype(np.uint16)).view(np.float16
                                                            ).astype(np.float64)
        dB = pred[p + 1, 0].ravel().astype(np.float64)
        dbh = dB[widx]
        S = wz.sum() - dbh.sum()
        hits = len(widx)
        cnt = nbcount[p + 1] + hits - int(np.count_nonzero(dbh))
        total += S / max(cnt, 1.0)
    return np.float32(total)


# revision 10
# speedup vs baseline: 3.3951x; 3.3951x over previous
"""ConsistencyLoss Trainium2 kernel (single device phase).

Problem: B=16 depth frames, 15 consecutive pairs. Per pair: unproject
depth A, rigid-transform into frame B, project+round, z-buffer
scatter-min into B's grid, compare with depth B -> scalar loss; summed.

Sharding: data-parallel over pairs, 2 pairs per core across 8 cores
(core 7 duplicates pair 13 in slot 0; host ignores it).

Device (one launch, per core, 12 row-chunks of 128x1024):
with r = 1/d, the projection is u2 = Nx/Nz, v2 = Ny/Nz, z = d*Nz where
  N_* = A_*.u + c_*(v) + T_*.r      (affine field + reciprocal term)
Holes (d=0) self-eliminate: r=inf -> Nz=+-inf -> u2=NaN/huge -> the
rounded u16 coord falls outside [1,1024]. z<=0 is killed by relu(Nz)
before the 1/Nz reciprocal (1/0=inf -> coords saturate out of range).
Rounding uses the +-2^23 RNE trick (matches jnp.round); coords are
emitted +1-shifted as fp16 (exact for integers <=2048), z as fp16.
Work is split DVE / Activation / GPSIMD roughly 7/5/6 us per chunk.
Per-frame nonzero counts (cnt denominators) come from an
is_finite(1/d) accumulation on the Activation engine.

Host: the per-pair scatter-min combine - u64 key sort ((idx<<16)|zbits,
fp16 bit order = value order for z>0) - plus the O(hits) loss assembly
S = sum(zmin) - sum(dB[hit]), cnt = nb(dB) + hits - nb_hit. This stays
on the host because TRN2 has no per-element scatter primitive
(indirect DMA RMW races lose duplicate updates; gpsimd scatter ops
share indices across partitions).
"""
import os
import sys

try:
    import concourse.bass as bass
except ImportError:
    sys.path.insert(0, "/opt/trn_rl_repo")
    import concourse.bass as bass

import numpy as np
import concourse.mybir as mybir
from concourse.bass_utils import run_bass_kernel_spmd

f32 = mybir.dt.float32
f16 = mybir.dt.float16
Alu = mybir.AluOpType
Act = mybir.ActivationFunctionType

B, H, W = 16, 768, 1024
NPAIR = B - 1          # 15
NCORE = 8
CHUNKS = H // 128      # 6
NCH = 2 * CHUNKS       # 12
M23 = float(1.5 * 2.0 ** 23)     # RNE rounding magic
BIAS1 = M23 + 1.0                # round + 1-shift in one add

LAST_PROFILE = {}


def _trace_enabled():
    return os.environ.get("CONSISTENCY_TRACE", "0") == "1"


def _quat_to_rot(q):
    q = q / np.linalg.norm(q)
    x, y, z, w = q
    return np.array([
        [1 - 2 * (y * y + z * z), 2 * (x * y - z * w), 2 * (x * z + y * w)],
        [2 * (x * y + z * w), 1 - 2 * (x * x + z * z), 2 * (y * z - x * w)],
        [2 * (x * z - y * w), 2 * (y * z + x * w), 1 - 2 * (x * x + y * y)],
    ])


def build_kernel():
    nc = bass.Bass()
    frames = nc.declare_dram_parameter("frames", [2, H, W], f32, isOutput=False)
    coefs = nc.declare_dram_parameter("coefs", [128, 49], f32, isOutput=False)
    uramp = nc.declare_dram_parameter("uramp", [128, W], f32, isOutput=False)
    out = nc.declare_dram_parameter("out", [2, H, 3 * W], f16, isOutput=True)
    nbacc = nc.declare_dram_parameter("nbacc", [128, NCH], f32, isOutput=True)

    import contextlib
    with contextlib.ExitStack() as stack:
        en = stack.enter_context
        d = en(nc.sbuf_tensor([128, 4 * W], f32))      # input depth, 4 slots
        rb = en(nc.sbuf_tensor([128, 2 * W], f32))     # 1/d (ACT), 2 slots
        nzb = en(nc.sbuf_tensor([128, 2 * W], f32))    # Nz
        nzpb = en(nc.sbuf_tensor([128, 2 * W], f32))   # relu(Nz)
        u2s = en(nc.sbuf_tensor([128, W], f32))        # Nx/Nz (DVE only)
        nyb = en(nc.sbuf_tensor([128, 2 * W], f32))    # Ny
        rzb = en(nc.sbuf_tensor([128, 2 * W], f32))    # 1/relu(Nz)
        fzs = en(nc.sbuf_tensor([128, W], f32))        # field z (DVE only)
        fxb = en(nc.sbuf_tensor([128, 2 * W], f32))    # field x
        fyb = en(nc.sbuf_tensor([128, 2 * W], f32))    # field y
        nxs = en(nc.sbuf_tensor([128, W], f32))        # Nx (DVE only)
        v2b = en(nc.sbuf_tensor([128, 2 * W], f32))    # v2 (Pool -> DVE)
        nzps = en(nc.sbuf_tensor([128, W], f32))       # relu(Nz) (ACT only)
        scr = en(nc.sbuf_tensor([128, W], f32))        # ACT dump
        ob = en(nc.sbuf_tensor([128, 2 * 3 * W], f16))  # out tile, 2 slots
        ur = en(nc.sbuf_tensor([128, W], f32))         # u ramp
        co = en(nc.sbuf_tensor([128, 49], f32))        # coefficients
        nb = en(nc.sbuf_tensor([128, NCH], f32))       # nonzero count accum
        dsem = en(nc.semaphore())
        osem = en(nc.semaphore())
        vsem = en(nc.semaphore())
        asem = en(nc.semaphore())
        psem = en(nc.semaphore())
        block = en(nc.Block())
        def dsl(k):
            b = (k % 4) * W
            return d[:, b:b + W]

        def sl(t, k, w=W):
            b = (k % 2) * w
            return t[:, b:b + w]

        def orow(k, i):
            b = (k % 2) * 3 * W + i * W
            return ob[:, b:b + W]

        def ccol(t, s, i):
            c = 24 * s + i
            return t[:, c:c + 1]

        # coefficient columns per pair s: 0-5 czv, 6-11 cxv, 12-17 cyv,
        # 18 Az, 19 Ax, 20 Ay, 21 tz, 22 TX, 23 TY; col 48 = -1e-20
        # sem: vsem (DVE): Nz@4k+1 Ny@4k+2 uc@4k+3 vc@4k+4
        #      asem (ACT): pre 4; r@5k+5 nb@5k+6 rz@5k+7 fx@5k+8 fy@5k+9
        #      psem (Pool): v2@2k+1 zt@2k+2; dsem: d[k] at 16(k+3); osem: stores
        def act_recip(out_ap, in_ap, bias=0.0):
            eng = nc.scalar
            ins = [eng.lower_ap(in_ap)]
            for arg in (bias, 1.0, 0.0):
                ins.append(mybir.ImmediateValue(dtype=mybir.dt.float32, value=arg))
            return eng.add_instruction(mybir.InstActivation(
                name=nc.get_next_instruction_name(),
                func=Act.Reciprocal, ins=ins, outs=[eng.lower_ap(out_ap)]))

        @block.gpsimd
        def _(g):
            g.dma_start(ur[:], uramp[:]).then_inc(dsem, 16)
            g.dma_start(co[:], coefs[:]).then_inc(dsem, 16)
            for k in range(2):
                s, j = divmod(k, CHUNKS)
                g.dma_start(dsl(k), frames[s, 128 * j:128 * j + 128]
                            ).then_inc(dsem, 16)
            for k in range(NCH):
                s, j = divmod(k, CHUNKS)
                if k + 2 < NCH:
                    s2, j2 = divmod(k + 2, CHUNKS)
                    g.dma_start(dsl(k + 2), frames[s2, 128 * j2:128 * j2 + 128]
                                ).then_inc(dsem, 16)
                # v2 = Ny * (1/relu(Nz))
                g.wait_ge(asem, 5 * k + 7)
                g.wait_ge(vsem, 4 * k + 2)
                nc.gpsimd.tensor_tensor(
                    sl(v2b, k), sl(nyb, k), sl(rzb, k), Alu.mult).then_inc(psem, 1)
                # z = d * Nz -> fp16 (holes: d=-1e30 -> z<0 -> host drops)
                if k >= 2:
                    g.wait_ge(osem, 16 * (k - 1))
                nc.gpsimd.tensor_tensor(
                    orow(k, 2), dsl(k), sl(nzb, k), Alu.mult).then_inc(psem, 1)
                g.wait_ge(vsem, 4 * k + 4)
                g.dma_start(out[s, 128 * j:128 * j + 128], sl(ob, k, 3 * W)
                            ).then_inc(osem, 16)
            g.wait_ge(asem, 4 + 5 * NCH)
            g.dma_start(nbacc[:], nb[:]).then_inc(osem, 16)

        @block.vector
        def _(v):
            v.wait_ge(dsem, 32)
            for k in range(NCH):
                s, j = divmod(k, CHUNKS)
                nc.vector.tensor_scalar(
                    fzs[:], ur[:], ccol(co, s, 18), ccol(co, s, j),
                    Alu.mult, Alu.add)
                v.wait_ge(asem, 5 * k + 5)
                if k >= 2:
                    v.wait_ge(psem, 2 * (k - 2) + 2)
                nc.vector.scalar_tensor_tensor(
                    sl(nzb, k), sl(rb, k), ccol(co, s, 21), fzs[:],
                    Alu.mult, Alu.add).then_inc(vsem, 1)
                v.wait_ge(asem, 5 * k - 2 if k >= 2 else 2 * k + 1)
                nc.vector.scalar_tensor_tensor(
                    nxs[:], sl(rb, k), ccol(co, s, 22), sl(fxb, k),
                    Alu.mult, Alu.add)
                v.wait_ge(asem, 5 * k - 1 if k >= 2 else 2 * k + 2)
                nc.vector.scalar_tensor_tensor(
                    sl(nyb, k), sl(rb, k), ccol(co, s, 23), sl(fyb, k),
                    Alu.mult, Alu.add).then_inc(vsem, 1)
                v.wait_ge(asem, 5 * k + 7)
                nc.vector.tensor_tensor(
                    u2s[:], nxs[:], sl(rzb, k), Alu.mult)
                if k >= 2:
                    v.wait_ge(osem, 16 * (k - 1))
                nc.vector.tensor_scalar(
                    orow(k, 0), u2s[:], BIAS1, M23, Alu.add, Alu.subtract
                ).then_inc(vsem, 1)
                v.wait_ge(psem, 2 * k + 1)
                nc.vector.tensor_scalar(
                    orow(k, 1), sl(v2b, k), BIAS1, M23, Alu.add, Alu.subtract
                ).then_inc(vsem, 1)

        @block.scalar
        def _(a):
            a.wait_ge(dsem, 32)
            for k in range(2):
                s, j = divmod(k, CHUNKS)
                nc.scalar.activation(sl(fxb, k), ur[:], Act.Identity,
                                     bias=ccol(co, s, 6 + j), scale=ccol(co, s, 19)
                                     ).then_inc(asem, 1)
                nc.scalar.activation(sl(fyb, k), ur[:], Act.Identity,
                                     bias=ccol(co, s, 12 + j), scale=ccol(co, s, 20)
                                     ).then_inc(asem, 1)
            for k in range(NCH):
                s, j = divmod(k, CHUNKS)
                a.wait_ge(dsem, 16 * (k + 3))
                act_recip(sl(rb, k), dsl(k)).then_inc(asem, 1)
                # nonzero count: sign(d - eps) in {-1,+1}; host decodes
                nc.scalar.activation(scr[:], dsl(k), Act.Sign, bias=co[:, 48:49],
                                     accum_out=nb[:, k:k + 1]).then_inc(asem, 1)
                a.wait_ge(vsem, 4 * k + 1)
                nc.scalar.activation(nzps[:], sl(nzb, k), Act.Relu)
                act_recip(sl(rzb, k), nzps[:], bias=float(2.0 ** -42)).then_inc(asem, 1)
                kk = (k + 2) % NCH
                s3, j3 = divmod(kk, CHUNKS)
                a.wait_ge(vsem, 4 * k + 2)
                nc.scalar.activation(sl(fxb, kk), ur[:], Act.Identity,
                                     bias=ccol(co, s3, 6 + j3), scale=ccol(co, s3, 19)
                                     ).then_inc(asem, 1)
                nc.scalar.activation(sl(fyb, kk), ur[:], Act.Identity,
                                     bias=ccol(co, s3, 12 + j3), scale=ccol(co, s3, 20)
                                     ).then_inc(asem, 1)
    return nc


_NC = None


def _get_module():
    global _NC
    if _NC is None:
        _NC = build_kernel()
    return _NC


def _maybe_enable_hook():
    """Register the axon NTFF profile hook if the image lacks antenv."""
    if not _trace_enabled():
        return
    try:
        import types
        import antenv.axon_hooks  # noqa: F401
    except ImportError:
        try:
            import trn_agent_boot.trn_boot as tb
            hook = tb._ntff_profile_via_ctypes("/opt/axon/libaxon_pjrt.so")
            m = types.ModuleType("antenv.axon_hooks")
            m.get_axon_ntff_profile_hook = lambda: hook
            m.set_axon_ntff_profile_hook = lambda h: None
            pkg = sys.modules.get("antenv") or types.ModuleType("antenv")
            pkg.axon_hooks = m
            sys.modules.setdefault("antenv", pkg)
            sys.modules["antenv.axon_hooks"] = m
            import concourse.bass_utils as bu
            bu.upload_artifacts = lambda d: "local://" + str(d)
        except Exception:
            pass


STARTS = [0, 2, 4, 6, 8, 10, 12, 13]


def _make_coefs(pose, K):
    fx, fy, cx, cy = (float(K[0, 0]), float(K[1, 1]),
                      float(K[0, 2]), float(K[1, 2]))
    v = np.arange(H, dtype=np.float64)
    b_v = (v - cy) / fy
    all_coefs = []
    for c in range(NCORE):
        st = STARTS[c]
        co = np.zeros((128, 49), np.float32)
        co[:, 48] = np.float32(-1e-20)
        for s in range(2):
            i = st + s
            RA = _quat_to_rot(pose[i, 3:].astype(np.float64))
            tA = pose[i, :3].astype(np.float64)
            RB = _quat_to_rot(pose[i + 1, 3:].astype(np.float64))
            tB = pose[i + 1, :3].astype(np.float64)
            M = RB.T @ RA
            tp = RB.T @ (tA - tB)
            rows = {
                'z': (M[2, 0], M[2, 1], M[2, 2], tp[2]),
                'x': (fx * M[0, 0] + cx * M[2, 0], fx * M[0, 1] + cx * M[2, 1],
                      fx * M[0, 2] + cx * M[2, 2], fx * tp[0] + cx * tp[2]),
                'y': (fy * M[1, 0] + cy * M[2, 0], fy * M[1, 1] + cy * M[2, 1],
                      fy * M[1, 2] + cy * M[2, 2], fy * tp[1] + cy * tp[2]),
            }
            for gi, key in enumerate(('z', 'x', 'y')):
                C0, C1, C2, C3 = rows[key]
                colv = (-C0 * cx / fx + C1 * b_v + C2).astype(np.float32)
                for j in range(CHUNKS):
                    co[:, 24 * s + 6 * gi + j] = colv[128 * j:128 * (j + 1)]
                co[:, 24 * s + 18 + gi] = np.float32(C0 / fx)
                co[:, 24 * s + 21 + gi] = np.float32(C3)
        all_coefs.append(co)
    return all_coefs


def kernel(pred, pose, K):
    pred = np.asarray(pred, dtype=np.float32)
    pose = np.asarray(pose, dtype=np.float32)
    K = np.asarray(K, dtype=np.float32)

    _maybe_enable_hook()
    nc = _get_module()

    all_coefs = _make_coefs(pose, K)
    urnp = np.broadcast_to(np.arange(W, dtype=np.float32), (128, W)).copy()
    in_maps = []
    for c in range(NCORE):
        st = STARTS[c]
        f2 = np.ascontiguousarray(pred[st:st + 2, 0])
        in_maps.append({
            "frames": np.where(f2 == 0.0, np.float32(-1e9), f2),
            "coefs": all_coefs[c],
            "uramp": urnp,
        })

    res = run_bass_kernel_spmd(nc, in_maps, list(range(NCORE)),
                               trace=_trace_enabled())
    if res.exec_time_ns is not None:
        LAST_PROFILE["exec_ns"] = res.exec_time_ns

    # per-frame nonzero counts (frame f as some core's dA slot)
    nbcount = {}
    for c in range(NCORE):
        a = res.results[c]["nbacc"]
        for s in range(2):
            sg = float(a[:, 6 * s:6 * (s + 1)].sum(dtype=np.float64))
            nbcount[STARTS[c] + s] = (sg + float(H * W)) / 2.0
    nbcount[B - 1] = float(np.count_nonzero(pred[B - 1, 0]))

    total = 0.0
    for p in range(NPAIR):
        if p == 14:
            c, s = 7, 1
        else:
            c, s = p // 2, p % 2
        o = res.results[c]["out"][s]          # [H, 3W] fp16
        uc = o[:, 0:W]
        vc = o[:, W:2 * W]
        zb = o[:, 2 * W:3 * W]
        with np.errstate(invalid='ignore'):
            ui = uc.astype(np.float64)
            vi = vc.astype(np.float64)
        zbits = zb.view(np.uint16).astype(np.int64)
        ok = ((ui >= 1) & (ui <= W) & (vi >= 1) & (vi <= H)
              & (zbits < 0x7C00) & (zbits > 0))
        idx = ((vi[ok] - 1).astype(np.int64) * W + (ui[ok] - 1).astype(np.int64))
        key = (idx << 16) | zbits[ok]
        key.sort()
        kidx = key >> 16
        first = np.ones(len(key), bool)
        first[1:] = kidx[1:] != kidx[:-1]
        widx = kidx[first]
        wz = ((key[first] & 0xFFFF).astype(np.uint16)).view(np.float16
                                                            ).astype(np.float64)
        dB = pred[p + 1, 0].ravel().astype(np.float64)
        dbh = dB[widx]
        S = wz.sum() - dbh.sum()
        hits = len(widx)
        cnt = nbcount[p + 1] + hits - int(np.count_nonzero(dbh))
        total += S / max(cnt, 1.0)
    return np.float32(total)


# revision 12
# speedup vs baseline: 3.4547x; 1.0176x over previous
"""ConsistencyLoss Trainium2 kernel (single device phase).

Problem: B=16 depth frames, 15 consecutive pairs. Per pair: unproject
depth A, rigid-transform into frame B, project+round, z-buffer
scatter-min into B's grid, compare with depth B -> scalar loss; summed.

Sharding: data-parallel over pairs, 2 pairs per core across 8 cores
(core 7 duplicates pair 13 in slot 0; host ignores it).

Device (one launch, per core, 12 row-chunks of 128x1024):
with r = 1/d, the projection is u2 = Nx/Nz, v2 = Ny/Nz, z = d*Nz where
  N_* = A_*.u + c_*(v) + T_*.r      (affine field + reciprocal term)
Holes (d=0) self-eliminate: r=inf -> Nz=+-inf -> u2=NaN/huge -> the
rounded u16 coord falls outside [1,1024]. z<=0 is killed by relu(Nz)
before the 1/Nz reciprocal (1/0=inf -> coords saturate out of range).
Rounding uses the +-2^23 RNE trick (matches jnp.round); coords are
emitted +1-shifted as fp16 (exact for integers <=2048), z as fp16.
Work is split DVE / Activation / GPSIMD roughly 7/5/6 us per chunk.
Per-frame nonzero counts (cnt denominators) come from an
is_finite(1/d) accumulation on the Activation engine.

Host: the per-pair scatter-min combine - u64 key sort ((idx<<16)|zbits,
fp16 bit order = value order for z>0) - plus the O(hits) loss assembly
S = sum(zmin) - sum(dB[hit]), cnt = nb(dB) + hits - nb_hit. This stays
on the host because TRN2 has no per-element scatter primitive
(indirect DMA RMW races lose duplicate updates; gpsimd scatter ops
share indices across partitions).
"""
import os
import sys

try:
    import concourse.bass as bass
except ImportError:
    sys.path.insert(0, "/opt/trn_rl_repo")
    import concourse.bass as bass

import numpy as np
import concourse.mybir as mybir
from concourse.bass_utils import run_bass_kernel_spmd

f32 = mybir.dt.float32
f16 = mybir.dt.float16
Alu = mybir.AluOpType
Act = mybir.ActivationFunctionType

B, H, W = 16, 768, 1024
NPAIR = B - 1          # 15
NCORE = 8
CHUNKS = H // 128      # 6
NCH = 2 * CHUNKS       # 12
M23 = float(1.5 * 2.0 ** 23)     # RNE rounding magic
BIAS1 = M23 + 1.0                # round + 1-shift in one add

LAST_PROFILE = {}


def _trace_enabled():
    return os.environ.get("CONSISTENCY_TRACE", "0") == "1"


def _quat_to_rot(q):
    q = q / np.linalg.norm(q)
    x, y, z, w = q
    return np.array([
        [1 - 2 * (y * y + z * z), 2 * (x * y - z * w), 2 * (x * z + y * w)],
        [2 * (x * y + z * w), 1 - 2 * (x * x + z * z), 2 * (y * z - x * w)],
        [2 * (x * z - y * w), 2 * (y * z + x * w), 1 - 2 * (x * x + y * y)],
    ])


def build_kernel():
    nc = bass.Bass()
    frames = nc.declare_dram_parameter("frames", [2, H, W], f32, isOutput=False)
    coefs = nc.declare_dram_parameter("coefs", [128, 49], f32, isOutput=False)
    uramp = nc.declare_dram_parameter("uramp", [128, W], f32, isOutput=False)
    ouv = nc.declare_dram_parameter("ouv", [2, H, 2 * W], f32, isOutput=True)
    oz = nc.declare_dram_parameter("oz", [2, H, W], f16, isOutput=True)

    import contextlib
    with contextlib.ExitStack() as stack:
        en = stack.enter_context
        d = en(nc.sbuf_tensor([128, 4 * W], f32))      # input depth, 4 slots
        rb = en(nc.sbuf_tensor([128, 2 * W], f32))     # 1/d (ACT), 2 slots
        nzb = en(nc.sbuf_tensor([128, 2 * W], f32))    # Nz
        nyb = en(nc.sbuf_tensor([128, 2 * W], f32))    # Ny
        rzb = en(nc.sbuf_tensor([128, 2 * W], f32))    # 1/Nz (ACT)
        v2b = en(nc.sbuf_tensor([128, 2 * W], f32))    # v2 (Pool -> DVE)
        fzb = en(nc.sbuf_tensor([128, 2 * W], f32))    # field z (ACT)
        fxb = en(nc.sbuf_tensor([128, 2 * W], f32))    # field x (ACT)
        fyb = en(nc.sbuf_tensor([128, 2 * W], f32))    # field y (ACT)
        nxs = en(nc.sbuf_tensor([128, W], f32))        # Nx (DVE only)
        u2s = en(nc.sbuf_tensor([128, W], f32))        # u2 (DVE only)
        obu = en(nc.sbuf_tensor([128, 2 * 2 * W], f32))  # uc|vc tile, 2 slots
        obz = en(nc.sbuf_tensor([128, 2 * W], f16))    # z tile, 2 slots
        ur = en(nc.sbuf_tensor([128, W], f32))         # u ramp
        co = en(nc.sbuf_tensor([128, 49], f32))        # coefficients
        dsem = en(nc.semaphore())
        osem = en(nc.semaphore())
        vsem = en(nc.semaphore())
        asem = en(nc.semaphore())
        psem = en(nc.semaphore())
        block = en(nc.Block())

        def dsl(k):
            b = (k % 4) * W
            return d[:, b:b + W]

        def sl(t, k, w=W):
            b = (k % 2) * w
            return t[:, b:b + w]

        def uvrow(k, i):
            b = (k % 2) * 2 * W + i * W
            return obu[:, b:b + W]

        def ccol(t, s, i):
            c = 24 * s + i
            return t[:, c:c + 1]

        # coefficient columns per pair s: 0-5 czv, 6-11 cxv, 12-17 cyv,
        # 18 Az, 19 Ax, 20 Ay, 21 tz, 22 TX, 23 TY
        # sem: vsem (DVE): Nz@4k+1 Ny@4k+2 uc@4k+3 vc@4k+4
        #      asem (ACT): pre 6; r@5k+7 rz@5k+8 f*@5k+9..11
        #      psem (Pool): v2@2k+1 zt@2k+2; dsem: d[k] at 16(k+3)
        #      osem: 2 stores/chunk -> chunk k done at 32(k+1)
        def act_recip(out_ap, in_ap, bias=0.0):
            eng = nc.scalar
            ins = [eng.lower_ap(in_ap)]
            for arg in (bias, 1.0, 0.0):
                ins.append(mybir.ImmediateValue(dtype=mybir.dt.float32, value=arg))
            return eng.add_instruction(mybir.InstActivation(
                name=nc.get_next_instruction_name(),
                func=Act.Reciprocal, ins=ins, outs=[eng.lower_ap(out_ap)]))

        @block.gpsimd
        def _(g):
            g.dma_start(ur[:], uramp[:]).then_inc(dsem, 16)
            g.dma_start(co[:], coefs[:]).then_inc(dsem, 16)
            for k in range(2):
                s, j = divmod(k, CHUNKS)
                g.dma_start(dsl(k), frames[s, 128 * j:128 * j + 128]
                            ).then_inc(dsem, 16)
            for k in range(NCH):
                s, j = divmod(k, CHUNKS)
                if k + 2 < NCH:
                    s2, j2 = divmod(k + 2, CHUNKS)
                    g.dma_start(dsl(k + 2), frames[s2, 128 * j2:128 * j2 + 128]
                                ).then_inc(dsem, 16)
                # v2 = Ny / Nz
                g.wait_ge(asem, 5 * k + 8)
                g.wait_ge(vsem, 4 * k + 2)
                nc.gpsimd.tensor_tensor(
                    sl(v2b, k), sl(nyb, k), sl(rzb, k), Alu.mult).then_inc(psem, 1)
                # z = d * Nz -> fp16 (holes d=-1e9 and z<0 both -> host drops)
                if k >= 2:
                    g.wait_ge(osem, 32 * (k - 1))
                nc.gpsimd.tensor_tensor(
                    sl(obz, k), dsl(k), sl(nzb, k), Alu.mult).then_inc(psem, 1)
                g.wait_ge(vsem, 4 * k + 4)
                g.dma_start(ouv[s, 128 * j:128 * j + 128], sl(obu, k, 2 * W)
                            ).then_inc(osem, 16)
                g.dma_start(oz[s, 128 * j:128 * j + 128], sl(obz, k)
                            ).then_inc(osem, 16)

        @block.vector
        def _(v):
            for k in range(NCH):
                s, j = divmod(k, CHUNKS)
                v.wait_ge(asem, 5 * k + 7)
                if k >= 2:
                    v.wait_ge(psem, 2 * (k - 2) + 2)
                nc.vector.scalar_tensor_tensor(
                    sl(nzb, k), sl(rb, k), ccol(co, s, 21), sl(fzb, k),
                    Alu.mult, Alu.add).then_inc(vsem, 1)
                nc.vector.scalar_tensor_tensor(
                    nxs[:], sl(rb, k), ccol(co, s, 22), sl(fxb, k),
                    Alu.mult, Alu.add)
                nc.vector.scalar_tensor_tensor(
                    sl(nyb, k), sl(rb, k), ccol(co, s, 23), sl(fyb, k),
                    Alu.mult, Alu.add).then_inc(vsem, 1)
                v.wait_ge(asem, 5 * k + 8)
                nc.vector.tensor_tensor(
                    u2s[:], nxs[:], sl(rzb, k), Alu.mult)
                # uc/vc = round(x)+1 in f32 (RNE via +-2^23); host filters
                if k >= 2:
                    v.wait_ge(osem, 32 * (k - 1))
                nc.vector.tensor_scalar(
                    uvrow(k, 0), u2s[:], BIAS1, M23, Alu.add, Alu.subtract
                ).then_inc(vsem, 1)
                v.wait_ge(psem, 2 * k + 1)
                nc.vector.tensor_scalar(
                    uvrow(k, 1), sl(v2b, k), BIAS1, M23, Alu.add, Alu.subtract
                ).then_inc(vsem, 1)

        @block.scalar
        def _(a):
            a.wait_ge(dsem, 32)
            for k in range(2):
                s, j = divmod(k, CHUNKS)
                nc.scalar.activation(sl(fzb, k), ur[:], Act.Identity,
                                     bias=ccol(co, s, j), scale=ccol(co, s, 18)
                                     ).then_inc(asem, 1)
                nc.scalar.activation(sl(fxb, k), ur[:], Act.Identity,
                                     bias=ccol(co, s, 6 + j), scale=ccol(co, s, 19)
                                     ).then_inc(asem, 1)
                nc.scalar.activation(sl(fyb, k), ur[:], Act.Identity,
                                     bias=ccol(co, s, 12 + j), scale=ccol(co, s, 20)
                                     ).then_inc(asem, 1)
            for k in range(NCH):
                a.wait_ge(dsem, 16 * (k + 3))
                act_recip(sl(rb, k), dsl(k)).then_inc(asem, 1)
                a.wait_ge(vsem, 4 * k + 1)
                act_recip(sl(rzb, k), sl(nzb, k)).then_inc(asem, 1)
                kk = (k + 2) % NCH
                s3, j3 = divmod(kk, CHUNKS)
                a.wait_ge(vsem, 4 * k + 2)
                nc.scalar.activation(sl(fzb, kk), ur[:], Act.Identity,
                                     bias=ccol(co, s3, j3), scale=ccol(co, s3, 18)
                                     ).then_inc(asem, 1)
                nc.scalar.activation(sl(fxb, kk), ur[:], Act.Identity,
                                     bias=ccol(co, s3, 6 + j3), scale=ccol(co, s3, 19)
                                     ).then_inc(asem, 1)
                nc.scalar.activation(sl(fyb, kk), ur[:], Act.Identity,
                                     bias=ccol(co, s3, 12 + j3), scale=ccol(co, s3, 20)
                                     ).then_inc(asem, 1)
    return nc


_NC = None


def _get_module():
    global _NC
    if _NC is None:
        _NC = build_kernel()
    return _NC


def _maybe_enable_hook():
    """Register the axon NTFF profile hook if the image lacks antenv."""
    if not _trace_enabled():
        return
    try:
        import types
        import antenv.axon_hooks  # noqa: F401
    except ImportError:
        try:
            import trn_agent_boot.trn_boot as tb
            hook = tb._ntff_profile_via_ctypes("/opt/axon/libaxon_pjrt.so")
            m = types.ModuleType("antenv.axon_hooks")
            m.get_axon_ntff_profile_hook = lambda: hook
            m.set_axon_ntff_profile_hook = lambda h: None
            pkg = sys.modules.get("antenv") or types.ModuleType("antenv")
            pkg.axon_hooks = m
            sys.modules.setdefault("antenv", pkg)
            sys.modules["antenv.axon_hooks"] = m
            import concourse.bass_utils as bu
            bu.upload_artifacts = lambda d: "local://" + str(d)
        except Exception:
            pass


STARTS = [0, 2, 4, 6, 8, 10, 12, 13]


def _make_coefs(pose, K):
    fx, fy, cx, cy = (float(K[0, 0]), float(K[1, 1]),
                      float(K[0, 2]), float(K[1, 2]))
    v = np.arange(H, dtype=np.float64)
    b_v = (v - cy) / fy
    all_coefs = []
    for c in range(NCORE):
        st = STARTS[c]
        co = np.zeros((128, 49), np.float32)
        co[:, 48] = np.float32(-1e-20)
        for s in range(2):
            i = st + s
            RA = _quat_to_rot(pose[i, 3:].astype(np.float64))
            tA = pose[i, :3].astype(np.float64)
            RB = _quat_to_rot(pose[i + 1, 3:].astype(np.float64))
            tB = pose[i + 1, :3].astype(np.float64)
            M = RB.T @ RA
            tp = RB.T @ (tA - tB)
            rows = {
                'z': (M[2, 0], M[2, 1], M[2, 2], tp[2]),
                'x': (fx * M[0, 0] + cx * M[2, 0], fx * M[0, 1] + cx * M[2, 1],
                      fx * M[0, 2] + cx * M[2, 2], fx * tp[0] + cx * tp[2]),
                'y': (fy * M[1, 0] + cy * M[2, 0], fy * M[1, 1] + cy * M[2, 1],
                      fy * M[1, 2] + cy * M[2, 2], fy * tp[1] + cy * tp[2]),
            }
            for gi, key in enumerate(('z', 'x', 'y')):
                C0, C1, C2, C3 = rows[key]
                colv = (-C0 * cx / fx + C1 * b_v + C2).astype(np.float32)
                for j in range(CHUNKS):
                    co[:, 24 * s + 6 * gi + j] = colv[128 * j:128 * (j + 1)]
                co[:, 24 * s + 18 + gi] = np.float32(C0 / fx)
                co[:, 24 * s + 21 + gi] = np.float32(C3)
        all_coefs.append(co)
    return all_coefs


def kernel(pred, pose, K):
    pred = np.asarray(pred, dtype=np.float32)
    pose = np.asarray(pose, dtype=np.float32)
    K = np.asarray(K, dtype=np.float32)

    _maybe_enable_hook()
    nc = _get_module()

    all_coefs = _make_coefs(pose, K)
    urnp = np.broadcast_to(np.arange(W, dtype=np.float32), (128, W)).copy()
    in_maps = []
    for c in range(NCORE):
        st = STARTS[c]
        f2 = np.ascontiguousarray(pred[st:st + 2, 0])
        in_maps.append({
            "frames": np.where(f2 == 0.0, np.float32(-1e9), f2),
            "coefs": all_coefs[c],
            "uramp": urnp,
        })

    res = run_bass_kernel_spmd(nc, in_maps, list(range(NCORE)),
                               trace=_trace_enabled())
    if res.exec_time_ns is not None:
        LAST_PROFILE["exec_ns"] = res.exec_time_ns

    total = 0.0
    for p in range(NPAIR):
        if p == 14:
            c, s = 7, 1
        else:
            c, s = p // 2, p % 2
        uv = res.results[c]["ouv"][s]         # [H, 2W] f32
        zb = res.results[c]["oz"][s]          # [H, W] fp16
        ui = uv[:, 0:W].astype(np.float64)
        vi = uv[:, W:2 * W].astype(np.float64)
        zbits = zb.view(np.uint16).astype(np.int64)
        ok = ((ui >= 1) & (ui <= W) & (vi >= 1) & (vi <= H)
              & (zbits < 0x7C00) & (zbits > 0))
        idx = ((vi[ok] - 1).astype(np.int64) * W + (ui[ok] - 1).astype(np.int64))
        key = (idx << 16) | zbits[ok]
        key.sort()
        kidx = key >> 16
        first = np.ones(len(key), bool)
        first[1:] = kidx[1:] != kidx[:-1]
        widx = kidx[first]
        wz = ((key[first] & 0xFFFF).astype(np.uint16)).view(np.float16
                                                            ).astype(np.float64)
        dB = pred[p + 1, 0].ravel().astype(np.float64)
        dbh = dB[widx]
        S = wz.sum() - dbh.sum()
        hits = len(widx)
        cnt = float(np.count_nonzero(dB)) + hits - int(np.count_nonzero(dbh))
        total += S / max(cnt, 1.0)
    return np.float32(total)
